# revision 20
# baseline (speedup 1.0000x reference)
"""Causal multi-head self-attention on 8 Trainium2 NeuronCores.

Problem: B=8, T=1024, D=1024, 16 heads (H=64), fp32 in/out, causal softmax,
y = softmax(mask(q k^T)/sqrt(H)) v @ Wo^T. Weights are nn.Linear style:
q = x @ Wq^T etc.

Sharding: pure data-parallel - one batch element per core, weights
replicated, no collectives.

Implementation notes (v2, bf16 compute):
  * All matmul operands stored bf16 (1 cycle/row on PE, half the DMA and
    SBUF footprint of fp32/f32r); PSUM accumulation is fp32; final y fp32.
  * Feature-major layouts avoid on-device transposes:
      xT [d,t], qT/kT [do,t], v natural [t,do] packed per head with a ones
      column (vp) so the attV matmul also produces softmax denominators.
  * Causal mask applied as a multiply by a 0/1 keep-mask on the exp()
    output (DVE), not as PE matmuls: exp(s + (-inf)*m) == exp(s)*keep.
  * Software pipeline over head pairs pr (=128-row feature tile):
      iter i: qk-proj(i+2) | scores(i+1)+exp+mask | attV(i)+normalize
    so ACT exp time hides under PE matmul time of the same iteration.
  * Warm-up matmuls on a zero tile during the input DMA keep the PE HAM
    clock-gate at 2.4 GHz (idle >3.4us drops it to 1.2 GHz).
  * Softmax normalization: denominator rows are DMA-gathered into a
    [16,128] tile, one batched reciprocal, DRAM-bounce broadcast back as
    a single [64,2048] stride-0 DMA per pair.
"""

import numpy as np
from contextlib import ExitStack

N_CORES = 8
T = 1024
D = 1024
NH = 16
HD = 64   # head dim
P = 128
NT = D // P  # 8 tiles of 128 along d or t
TQ = 512
VP = HD + 1  # per-head v columns + ones column
SCALE = 1.0 / 8.0  # 1/sqrt(64)
N_WARM = 40  # PE warm-up matmuls during input DMA

MM_MODE = "bf16"  # informational; this kernel always computes in bf16

_COMPILED = {}


def _build(nc, tile, mybir):
    f32 = mybir.dt.float32
    bf16 = mybir.dt.bfloat16
    Exp = mybir.ActivationFunctionType.Exp

    xT_d = nc.declare_dram_parameter("xT", [D, T], bf16, isOutput=False)
    wqT_d = nc.declare_dram_parameter("wqT", [D, D], bf16, isOutput=False)
    wkT_d = nc.declare_dram_parameter("wkT", [D, D], bf16, isOutput=False)
    wvT_d = nc.declare_dram_parameter("wvT", [D, D], bf16, isOutput=False)
    woT_d = nc.declare_dram_parameter("woT", [D, D], bf16, isOutput=False)
    kmask_d = nc.declare_dram_parameter("kmask", [P, 4 * TQ], bf16, isOutput=False)
    y_d = nc.declare_dram_parameter("y", [T, D], f32, isOutput=True)

    # flush scratch: per pair, 2048 reciprocal denominators (2 halves x 1024 tq)
    nrm_d = nc.dram_tensor("nrm_scratch", [NT, 2 * T], bf16)

    def mm(out, lhsT, rhs, start, stop):
        nc.tensor.matmul(out, lhsT, rhs, start=start, stop=stop)

    with ExitStack() as ctx:
        tc = ctx.enter_context(tile.TileContext(nc))

        # ---- pools (all resident, no phase barriers) ----
        pconst = ctx.enter_context(tc.tile_pool(name="pconst", bufs=2))
        pw = ctx.enter_context(tc.tile_pool(name="pw", bufs=24))
        pxt = ctx.enter_context(tc.tile_pool(name="pxt", bufs=8))
        pqk = ctx.enter_context(tc.tile_pool(name="pqk", bufs=16))
        pv = ctx.enter_context(tc.tile_pool(name="pv", bufs=8))
        pout = ctx.enter_context(tc.tile_pool(name="pout", bufs=8))
        pe = ctx.enter_context(tc.tile_pool(name="pe", bufs=14))
        py = ctx.enter_context(tc.tile_pool(name="py", bufs=3))
        pux = ctx.enter_context(tc.tile_pool(name="pux", bufs=4))
        pnt = ctx.enter_context(tc.tile_pool(name="pnt", bufs=2))
        pct = ctx.enter_context(tc.tile_pool(name="pct", bufs=2))
        pbc = ctx.enter_context(tc.tile_pool(name="pbc", bufs=1))
        # psum: 3 shared [128,1024] slots (2 banks each) + 1 attV slot (2) = 8
        pp = ctx.enter_context(tc.tile_pool(name="pp", bufs=3, space="PSUM"))
        pp_att = ctx.enter_context(
            tc.tile_pool(name="pp_att", bufs=1, space="PSUM")
        )

        # ---- constants / warm-up tile ----
        warm = pconst.tile([P, TQ], bf16)
        nc.vector.memset(warm[:], 0.0)
        kmask_sb = pconst.tile([P, 4 * TQ], bf16)
        nc.sync.dma_start(out=kmask_sb[:], in_=kmask_d[:])

        # ---- resident tensors ----
        wq_sb = [pw.tile([P, D], bf16, tag="w", name=f"wq{i}") for i in range(NT)]
        wk_sb = [pw.tile([P, D], bf16, tag="w", name=f"wk{i}") for i in range(NT)]
        wo_sb = [pw.tile([P, D], bf16, tag="w", name=f"wo{i}") for i in range(NT)]
        xT = [pxt.tile([P, T], bf16, tag="xt", name=f"xT{i}") for i in range(NT)]
        qT = [pqk.tile([P, T], bf16, tag="qk", name=f"qT{i}") for i in range(NT)]
        kT = [pqk.tile([P, T], bf16, tag="qk", name=f"kT{i}") for i in range(NT)]
        vp = [pv.tile([P, NH * VP], bf16, tag="vp", name=f"vp{i}")
              for i in range(NT)]
        outT = [pout.tile([P, T], bf16, tag="ot", name=f"outT{i}")
                for i in range(NT)]

        # v-weights live only through the preamble; their space is then
        # reused for extra e-tile buffers (pe2) below.
        with tc.tile_pool(name="pwv", bufs=8) as pwv:
            wv_sb = [pwv.tile([P, D], bf16, tag="wv", name=f"wv{i}")
                     for i in range(NT)]
            # ---- input DMAs (order matters: v-proj inputs first) ----
            for k in range(NT):
                nc.sync.dma_start(
                    out=wv_sb[k][:], in_=wvT_d[k * P : (k + 1) * P, :]
                )
                nc.sync.dma_start(out=xT[k][:], in_=xT_d[k * P : (k + 1) * P, :])
            for k in range(NT):
                nc.sync.dma_start(
                    out=wq_sb[k][:], in_=wqT_d[k * P : (k + 1) * P, :]
                )
            for k in range(NT):
                nc.sync.dma_start(
                    out=wk_sb[k][:], in_=wkT_d[k * P : (k + 1) * P, :]
                )
            for k in range(NT):
                nc.sync.dma_start(
                    out=wo_sb[k][:], in_=woT_d[k * P : (k + 1) * P, :]
                )

            # ones columns of v-plus
            for m in range(NT):
                ones_cols = vp[m].rearrange(
                    "p (h c) -> p h c", c=VP
                )[:, :, HD : HD + 1]
                nc.vector.memset(ones_cols, 1.0)

            # ---- PE warm-up: keep the HAM clock busy during input DMA ----
            wps = pp.tile([P, T], f32, tag="ps", name="warmps")
            for _ in range(N_WARM):
                mm(wps[:, 0:TQ], warm[:, 0:P], warm[:], start=True, stop=True)

            # ---- v projection (natural layout, scattered into vp) ----
            for m in range(NT):
                ps = pp.tile([P, T], f32, tag="ps")
                for k in range(NT):
                    for c in range(2):
                        mm(ps[:, c * TQ : (c + 1) * TQ],
                           xT[k][:, m * P : (m + 1) * P],
                           wv_sb[k][:, c * TQ : (c + 1) * TQ],
                           start=(k == 0), stop=(k == NT - 1))
                vdst = vp[m].rearrange("p (h c) -> p h c", c=VP)[:, :, 0:HD]
                vsrc = ps.rearrange("p (h c) -> p h c", c=HD)
                nc.vector.tensor_copy(vdst, vsrc)

        pe2 = ctx.enter_context(tc.tile_pool(name="pe2", bufs=8))
        e_state = {"n": 0}

        def e_tile():
            idx = e_state["n"]
            e_state["n"] = idx + 1
            pool = pe if idx % 22 < 14 else pe2
            return pool.tile([P, T], bf16, tag="e", name=f"e{idx}")

        def proj_qk(it):
            for w_sb, dst in ((wq_sb, qT), (wk_sb, kT)):
                ps = pp.tile([P, T], f32, tag="ps")
                for k in range(NT):
                    for c in range(2):
                        mm(ps[:, c * TQ : (c + 1) * TQ],
                           w_sb[k][:, it * P : (it + 1) * P],
                           xT[k][:, c * TQ : (c + 1) * TQ],
                           start=(k == 0), stop=(k == NT - 1))
                nc.vector.tensor_copy(dst[it][:], ps[:])

        def scores(pr):
            """Scores + exp + causal mask for head pair pr.

            Returns e tiles: elo[i][half] for i<4 covers tq 0:1024 of one
            half; ehi[i-4] for i>=4 covers tq 512:1024, cols packed
            (half0 | half1).
            """
            it = pr
            elo = [[None, None] for _ in range(4)]
            ehi = [None] * 4
            for i in range(4):
                g = i  # diagonal block offset for j0
                for half in range(2):
                    po = half * HD
                    ps = pp.tile([P, T], f32, tag="ps")
                    for c in range(2):
                        mm(ps[:, c * TQ : (c + 1) * TQ],
                           kT[it][po : po + HD, i * P : (i + 1) * P],
                           qT[it][po : po + HD, c * TQ : (c + 1) * TQ],
                           start=True, stop=True)
                    e = e_tile()
                    nc.scalar.activation(e[:], ps[:], Exp, scale=SCALE)
                    nc.vector.tensor_mul(
                        e[:, 0:TQ], e[:, 0:TQ],
                        kmask_sb[:, g * TQ : (g + 1) * TQ],
                    )
                    elo[i][half] = e
            for i in range(4, NT):
                g = i - 4
                ps = pp.tile([P, T], f32, tag="ps")
                for half in range(2):
                    po = half * HD
                    mm(ps[:, half * TQ : (half + 1) * TQ],
                       kT[it][po : po + HD, i * P : (i + 1) * P],
                       qT[it][po : po + HD, TQ:T], start=True, stop=True)
                e = e_tile()
                nc.scalar.activation(e[:], ps[:], Exp, scale=SCALE)
                for half in range(2):
                    nc.vector.tensor_mul(
                        e[:, half * TQ : (half + 1) * TQ],
                        e[:, half * TQ : (half + 1) * TQ],
                        kmask_sb[:, g * TQ : (g + 1) * TQ],
                    )
                ehi[i - 4] = e
            return elo, ehi

        def attv(pr, elo, ehi):
            """attV chains for both halves; returns (ux0, ux1)."""
            uxs = []
            for half in range(2):
                h0 = 2 * pr + half
                po_ps = pp_att.tile([VP, T], f32)
                for i in range(3):
                    for c in range(2):
                        mm(po_ps[:, c * TQ : (c + 1) * TQ],
                           vp[i][:, h0 * VP : (h0 + 1) * VP],
                           elo[i][half][:, c * TQ : (c + 1) * TQ],
                           start=(i == 0), stop=False)
                mm(po_ps[:, 0:TQ], vp[3][:, h0 * VP : (h0 + 1) * VP],
                   elo[3][half][:, 0:TQ], start=False, stop=True)
                mm(po_ps[:, TQ:T], vp[3][:, h0 * VP : (h0 + 1) * VP],
                   elo[3][half][:, TQ:T], start=False, stop=False)
                for i in range(4, NT):
                    mm(po_ps[:, TQ:T], vp[i][:, h0 * VP : (h0 + 1) * VP],
                       ehi[i - 4][:, half * TQ : (half + 1) * TQ],
                       start=False, stop=(i == NT - 1))
                ux = pux.tile([VP, T], bf16, tag="ux")
                nc.vector.tensor_copy(ux[:], po_ps[:])
                uxs.append(ux)
            return uxs

        def flush(pr, ux0, ux1):
            it = pr
            ct = pct.tile([16, P], bf16, tag="ct")
            nc.sync.dma_start(out=ct[0:8, :], in_=ux0[HD : HD + 1, :])
            nc.sync.dma_start(out=ct[8:16, :], in_=ux1[HD : HD + 1, :])
            with nc.allow_low_precision(reason="softmax denom recip in bf16"):
                nc.vector.reciprocal(ct[:], ct[:])
            nc.sync.dma_start(out=nrm_d[pr : pr + 1, :], in_=ct[:])
            bt = pbc.tile([HD, 2 * T], bf16, tag="bt")
            nc.sync.dma_start(
                out=bt[:], in_=nrm_d[pr : pr + 1, :].to_broadcast([HD, 2 * T])
            )
            nc.vector.tensor_mul(outT[it][0:HD, :], ux0[0:HD, :], bt[:, 0:T])
            nt_ = pnt.tile([HD, T], bf16, tag="nt")
            nc.vector.tensor_mul(nt_[:], ux1[0:HD, :], bt[:, T : 2 * T])
            nc.sync.dma_start(out=outT[it][HD:P, :], in_=nt_[:])

        # ---- pipelined head-pair loop ----
        proj_qk(0)
        proj_qk(1)
        sc = scores(0)
        for i in range(NT):
            if i + 2 < NT:
                proj_qk(i + 2)
            sc_next = scores(i + 1) if i + 1 < NT else None
            ux0, ux1 = attv(i, *sc)
            flush(i, ux0, ux1)
            sc = sc_next

        # ---- output projection ----
        # k=0..6 accumulate while the last pair's normalize (outT[7]) is
        # still in flight; the k=7 closers run two chains behind.
        def y_partial(m):
            ps = pp.tile([P, T], f32, tag="ps", name=f"yps{m}")
            for k in range(NT - 1):
                for c in range(2):
                    mm(ps[:, c * TQ : (c + 1) * TQ],
                       outT[k][:, m * P : (m + 1) * P],
                       wo_sb[k][:, c * TQ : (c + 1) * TQ],
                       start=(k == 0), stop=False)
            return ps

        def y_finish(m, ps):
            for c in range(2):
                mm(ps[:, c * TQ : (c + 1) * TQ],
                   outT[NT - 1][:, m * P : (m + 1) * P],
                   wo_sb[NT - 1][:, c * TQ : (c + 1) * TQ],
                   start=False, stop=True)
            for c in range(2):
                ysb = py.tile([P, TQ], f32, tag="y")
                nc.vector.tensor_copy(ysb[:], ps[:, c * TQ : (c + 1) * TQ])
                nc.sync.dma_start(
                    out=y_d[m * P : (m + 1) * P, c * TQ : (c + 1) * TQ],
                    in_=ysb[:],
                )

        yps = [None] * NT
        yps[0] = y_partial(0)
        yps[1] = y_partial(1)
        for m in range(NT):
            y_finish(m, yps[m])
            if m + 2 < NT:
                yps[m + 2] = y_partial(m + 2)
    return nc


def build_program(mm_mode=None):
    mode = "bf16"
    if mode in _COMPILED:
        return _COMPILED[mode]
    import concourse.bacc as bacc
    import concourse.tile as tile
    from concourse import mybir

    nc = bacc.Bacc("TRN2", target_bir_lowering=False, debug=False,
                   num_devices=N_CORES)
    _build(nc, tile, mybir)
    nc.compile()
    _COMPILED[mode] = nc
    return nc


def make_keepmask():
    # keep[p, g*512 + f] = 1 where kept (f >= p + 128*g), else 0
    import ml_dtypes
    p = np.arange(P)[:, None]
    f = np.arange(TQ)[None, :]
    cols = [(f >= p + P * g).astype(np.float32) for g in range(4)]
    return np.ascontiguousarray(
        np.concatenate(cols, axis=1).astype(ml_dtypes.bfloat16)
    )


def make_in_maps(x, Wk, Wq, Wv, Wo):
    import ml_dtypes
    bf = ml_dtypes.bfloat16
    x = np.asarray(x, dtype=np.float32)
    wqT = np.ascontiguousarray(np.asarray(Wq, dtype=np.float32).T.astype(bf))
    wkT = np.ascontiguousarray(np.asarray(Wk, dtype=np.float32).T.astype(bf))
    wvT = np.ascontiguousarray(np.asarray(Wv, dtype=np.float32).T.astype(bf))
    woT = np.ascontiguousarray(np.asarray(Wo, dtype=np.float32).T.astype(bf))
    kmask = make_keepmask()
    in_maps = []
    for b in range(N_CORES):
        in_maps.append({
            "xT": np.ascontiguousarray(x[b].T.astype(bf)),
            "wqT": wqT, "wkT": wkT, "wvT": wvT, "woT": woT,
            "kmask": kmask,
        })
    return in_maps


def kernel(x, Wk, Wq, Wv, Wo):
    from concourse.bass_utils import run_bass_kernel_spmd

    nc = build_program()
    in_maps = make_in_maps(x, Wk, Wq, Wv, Wo)
    res = run_bass_kernel_spmd(nc, in_maps, list(range(N_CORES)))
    return np.stack([res.results[c]["y"] for c in range(N_CORES)], axis=0)
